# revision 30
# baseline (speedup 1.0000x reference)
"""Trainium2 Bass kernel for nn_Net_58428735095641 (pairwise ResMLP + symmetric
scatter, training-mode BatchNorm with global batch stats).

Sharding: 8 cores = 4 batches x 2 halves of the 3240 pair rows (split at pair
1620). Each core runs the trunk on its 1620 rows in feature-major (transposed)
layout; the 5 BatchNorms need global row statistics, exchanged as raw
(sum, sumsq) via a tiny 8-core AllGather each. The final layer produces both
the 9x9 grid and its transpose pair-major, scattered into a per-core
(81*81, 81) output buffer with indirect DMAs; the host overlays the two
half-buffers per batch with plain slice assignments.
"""
import os
from contextlib import ExitStack

import numpy as np

import concourse.bass as bass
import concourse.mybir as mybir
import concourse.tile as tile
from concourse.vector_clock import ScopedClock

# ---------------------------------------------------------------------------
# Workarounds: the neuronxcc walrus on this path accepts at most ONE sync-wait
# per instruction; split extras onto same-engine nops.
# ---------------------------------------------------------------------------
MAX_WAITS = 1


def _drain_and_barrier(self, tick_clock, wait_clock):
    drain_inst = self.nc.sync.drain()
    wait_clock.add_sem_waits(
        drain_inst.ins, ScopedClock({None: tick_clock.global_clock})
    )
    waits = list(drain_inst.ins.sync_info.on_wait)
    if len(waits) > MAX_WAITS:
        drain_inst.ins.sync_info.on_wait = waits[:MAX_WAITS]
        for i in range(MAX_WAITS, len(waits), MAX_WAITS):
            nop = self.nc.sync.nop(hint="drain_wait_spill", nofuse=True)
            nop.ins.sync_info = mybir.SyncInfo(
                on_wait=waits[i : i + MAX_WAITS], on_update=[]
            )
    self.nc.all_engine_barrier()
    assert self.sems is not None
    popped = self.nc._tile_sem_poison_stack.pop()
    assert popped is self._sem_poison
    self.nc.clear_and_free_semaphores(list(self.sems.allocated().values()))
    self.nc.all_engine_barrier()


tile.TileContext._drain_and_barrier = _drain_and_barrier


def _split_multi_waits(nc):
    for fn in nc.m.functions:
        for bb in fn.blocks:
            insts = list(bb.instructions)
            out = []
            changed = False
            for inst in insts:
                si = inst.sync_info
                if si is not None and si.on_wait and len(si.on_wait) > MAX_WAITS:
                    waits = list(si.on_wait)
                    for j in range(MAX_WAITS, len(waits), MAX_WAITS):
                        out.append(
                            mybir.InstNoOp(
                                name=f"{inst.name}-wsplit{j}",
                                engine=inst.engine,
                                bass_nofuse=True,
                                sync_info=mybir.SyncInfo(
                                    on_wait=waits[j : j + MAX_WAITS], on_update=[]
                                ),
                            )
                        )
                    si.on_wait = waits[:MAX_WAITS]
                    changed = True
                out.append(inst)
            if changed:
                bb.instructions[:] = out


def _pipeline_scatter_waits(nc, depth=7):
    """The 27 indirect scatter DMAs write provably disjoint cells of the output,
    but Tile serializes them (WAW on the output tensor). Replace each op's
    DMASW-chain wait with its sibling's data-ready wait, keeping only a
    ring-capacity pipeline: op n waits for op n-depth's completion so at most
    `depth` ops (depth*128 descriptors <= the 1024-descriptor SWDGE ring) are
    in flight."""
    import copy as _copy

    idmas = []
    for fn in nc.m.functions:
        for bb in fn.blocks:
            for inst in bb.instructions:
                if (
                    type(inst).__name__ == "InstDMACopy"
                    and inst.engine == mybir.EngineType.Pool
                ):
                    idmas.append(inst)
    if not idmas:
        return
    lane_cum = {}
    hist = []
    for inst in idmas:
        upd = inst.sync_info.on_update[0]
        lane_cum[upd.ant_name] = lane_cum.get(upd.ant_name, 0) + upd.update_value
        hist.append((upd.ant_name, upd.id, lane_cum[upd.ant_name]))
    last_data = None
    for n, inst in enumerate(idmas):
        waits = [
            w for w in inst.sync_info.on_wait if not w.ant_name.startswith("DMASW")
        ]
        if waits:
            last_data = waits
        elif last_data is not None:
            waits = [_copy.copy(w) for w in last_data]
        if n >= depth:
            name, sid, val = hist[n - depth]
            w = _copy.copy((last_data or list(inst.sync_info.on_wait))[0])
            w.ant_name = name
            w.id = sid
            w.wait_value = val
            w.wait_mode = "sem-ge-imm"
            waits.append(w)
        inst.sync_info.on_wait = waits


# ---------------------------------------------------------------------------
# Problem constants (hardcoded from the reference)
# ---------------------------------------------------------------------------
GRID = 9
V = 81
NB_FT = 2
HID = 256
NBLOCKS = 2
EPS = 1e-5
BS = 4
NPAIR = 3240          # V*(V-1)/2
RC = 1620             # pair rows per core (half a batch)
NT = BS * NPAIR       # 12960 global rows for BN stats
NCORES = 8
GG = GRID * GRID      # 81
OUTROWS = V * V       # 6561 rows of 81 floats per batch

CHUNKS = [(0, 512), (512, 1024), (1024, 1536), (1536, 1620)]
NCH = len(CHUNKS)
PTILES = [(t * 128, min((t + 1) * 128, RC)) for t in range(13)]

F32 = mybir.dt.float32
F32R = mybir.dt.float32r
I32 = mybir.dt.int32
AF = mybir.ActivationFunctionType
ALU = mybir.AluOpType

_TI, _TJ = np.triu_indices(V, 1)


def _build_nc():
    nc = bass.Bass(num_devices=NCORES, num_swdge_queues=4)

    # fp32r inputs (matmul operands)
    d_rr = nc.dram_tensor("rrT", [4, RC], F32R, kind="ExternalInput")
    d_win = nc.dram_tensor("winT", [4, HID], F32R, kind="ExternalInput")
    d_wtr = nc.dram_tensor("wtrunk", [4, HID, HID], F32R, kind="ExternalInput")
    d_wout = nc.dram_tensor("woutT", [HID, 2 * GG], F32R, kind="ExternalInput")
    # fp32 params
    d_bin = nc.dram_tensor("b_in", [HID, 1], F32, kind="ExternalInput")
    d_g = nc.dram_tensor("gammas", [5, HID], F32, kind="ExternalInput")
    d_b = nc.dram_tensor("betas", [5, HID], F32, kind="ExternalInput")
    d_bout = nc.dram_tensor("bout_bc", [128, 2 * GG], F32, kind="ExternalInput")
    # scatter index tables
    d_uidx = nc.dram_tensor("uidx", [13, 128], I32, kind="ExternalInput")
    d_lidx = nc.dram_tensor("lidx", [13, 128], I32, kind="ExternalInput")
    d_didx = nc.dram_tensor("diag_idx", [128, 1], I32, kind="ExternalInput")
    # output
    d_out = nc.dram_tensor("out", [OUTROWS, GG], F32, kind="ExternalOutput")

    with tile.TileContext(nc) as tc, ExitStack() as ctx:
        cst = ctx.enter_context(tc.tile_pool(name="cst", bufs=1))
        act = ctx.enter_context(tc.tile_pool(name="act", bufs=2))
        sml = ctx.enter_context(tc.tile_pool(name="sml", bufs=2))
        ps = ctx.enter_context(tc.tile_pool(name="ps", bufs=8, space="PSUM"))
        dram = ctx.enter_context(tc.tile_pool(name="dram", bufs=2, space="DRAM"))

        # ---- inputs on the critical path to the first collective come first
        rr_sb = cst.tile([4, RC], F32R)
        nc.sync.dma_start(rr_sb[:], d_rr[:])
        win_sb = cst.tile([4, HID], F32R)
        nc.sync.dma_start(win_sb[:], d_win[:])
        bin_sb = cst.tile([128, 2], F32)
        nc.sync.dma_start(bin_sb[:], d_bin[:].rearrange("(m p) o -> p (m o)", p=128))
        g_sb = cst.tile([128, 10], F32)
        nc.sync.dma_start(g_sb[:], d_g[:].rearrange("n (m p) -> p (n m)", p=128))
        be_sb = cst.tile([128, 10], F32)
        nc.sync.dma_start(be_sb[:], d_b[:].rearrange("n (m p) -> p (n m)", p=128))

        def chunk_tiles(prefix, dtype=F32R, bufs=2):
            return [
                act.tile([128, c1 - c0], dtype, name=f"{prefix}_{ci}", bufs=bufs)
                for ci, (c0, c1) in enumerate(CHUNKS)
            ]

        # -------------------------------------------------------------------
        # Input layer: h0 = relu(W_in @ rrT + b_in); S accumulated by the relu
        # -------------------------------------------------------------------
        h = [chunk_tiles(f"h{m}", dtype=F32, bufs=3) for m in range(2)]
        spack0 = sml.tile([128, 4], F32, name="spack", bufs=3)
        sred = sml.tile([128, 2 * NCH], F32, name="sred")
        ssred = sml.tile([128, 2 * NCH], F32, name="ssred")
        for m in range(2):
            for ci, (c0, c1) in enumerate(CHUNKS):
                w = c1 - c0
                pz = ps.tile([128, 512], F32, tag="ps", name="pz")
                nc.tensor.matmul(
                    pz[:, :w],
                    lhsT=win_sb[:, m * 128 : (m + 1) * 128],
                    rhs=rr_sb[:, c0:c1],
                    start=True, stop=True,
                )
                nc.scalar.activation(
                    h[m][ci][:], pz[:, :w], AF.Relu,
                    bias=bin_sb[:, m : m + 1], scale=1.0,
                    accum_out=sred[:, m * NCH + ci : m * NCH + ci + 1],
                )
                scr = act.tile([128, 512], F32, name="sqscr5", bufs=3)
                if m == 0:
                    nc.scalar.activation(
                        scr[:, :w], h[m][ci][:], AF.Square,
                        accum_out=ssred[:, m * NCH + ci : m * NCH + ci + 1],
                    )
                else:
                    # DVE is idle during the input layer: square+reduce there
                    nc.vector.tensor_tensor(
                        out=scr[:, :w], in0=h[m][ci][:], in1=h[m][ci][:],
                        op=ALU.mult,
                    )
                    nc.vector.reduce_sum(
                        ssred[:, m * NCH + ci : m * NCH + ci + 1], scr[:, :w],
                        axis=mybir.AxisListType.X,
                    )
        for m in range(2):
            nc.vector.reduce_sum(
                spack0[:, m : m + 1], sred[:, m * NCH : (m + 1) * NCH],
                axis=mybir.AxisListType.X,
            )
            nc.vector.reduce_sum(
                spack0[:, 2 + m : 3 + m], ssred[:, m * NCH : (m + 1) * NCH],
                axis=mybir.AxisListType.X,
            )
        spack = spack0
        spack_h = spack0

        # ---- remaining constants (needed only after the first collective)
        w_sb = []  # [layer 0..3][kblock 0..1] -> [128, 256]
        for l in range(4):
            row = []
            for k in range(2):
                t = cst.tile([128, HID], F32R, name=f"w{l}{k}")
                nc.sync.dma_start(t[:], d_wtr[l, k * 128 : (k + 1) * 128, :])
                row.append(t)
            w_sb.append(row)
        wout_sb = []
        for k in range(2):
            t = cst.tile([128, 2 * GG], F32R, name=f"wout{k}")
            nc.sync.dma_start(t[:], d_wout[k * 128 : (k + 1) * 128, :])
            wout_sb.append(t)
        bout_sb = cst.tile([128, 2 * GG], F32)
        nc.sync.dma_start(bout_sb[:], d_bout[:])
        uidx_sb = cst.tile([128, 13], I32)
        nc.sync.dma_start(uidx_sb[:], d_uidx[:].rearrange("t p -> p t"))
        lidx_sb = cst.tile([128, 13], I32)
        nc.sync.dma_start(lidx_sb[:], d_lidx[:].rearrange("t p -> p t"))
        didx_sb = cst.tile([128, 1], I32)
        nc.sync.dma_start(didx_sb[:], d_didx[:])
        zer_sb = cst.tile([128, GG], F32)
        nc.vector.memset(zer_sb[:], 0.0)
        eps_sb = cst.tile([128, 1], F32)
        nc.vector.memset(eps_sb[:], EPS)
        # diagonal zero blocks depend only on constants: scatter them now,
        # while the Q7/SWDGE path is otherwise idle
        nc.gpsimd.indirect_dma_start(
            out=d_out[0:128, :],
            out_offset=bass.IndirectOffsetOnAxis(ap=didx_sb[:, 0:1], axis=0),
            in_=zer_sb[:],
            in_offset=None,
            bounds_check=OUTROWS - 1,
            oob_is_err=False,
        )

        # -------------------------------------------------------------------
        # Stats exchange + finalize. spack cols: (S_m0, S_m1, SS_m0, SS_m1).
        # -------------------------------------------------------------------
        def ag_and_finalize(spack, bn):
            """AllGather local (S, SS) and produce per-feature scale/shift."""
            agin = dram.tile([128, 4], F32, name="agin")
            agout = dram.tile([NCORES * 128, 4], F32, name="agout",
                              addr_space="Shared")
            nc.sync.dma_start(agin[:], spack[:])
            nc.gpsimd.collective_compute(
                "AllGather",
                ALU.bypass,
                replica_groups=[list(range(NCORES))],
                ins=[agin[:].opt()],
                outs=[agout[:].opt()],
            )
            gath = sml.tile([128, 32], F32, name="gath")
            nc.sync.dma_start(
                gath[:].rearrange("p (c r) -> p c r", r=8),
                agout[:].rearrange("(r p) c -> p c r", p=128),
            )
            gsum = sml.tile([128, 4], F32, name="gsum")
            nc.vector.reduce_sum(
                gsum[:], gath[:].rearrange("p (c r) -> p c r", r=8),
                axis=mybir.AxisListType.X,
            )
            # mean|ex2 in one op: cols (0:2)=S/NT, (2:4)=SS/NT
            mex = sml.tile([128, 4], F32, name="mex")
            nc.vector.tensor_scalar_mul(mex[:], gsum[:], 1.0 / NT)
            mean = mex[:, 0:2]
            var = sml.tile([128, 2], F32, name="var")
            nc.vector.tensor_tensor(out=var[:], in0=mean, in1=mean, op=ALU.mult)
            # var = ex2 - mean^2 + EPS via tensor_scalar two-op form
            nc.vector.tensor_tensor(out=var[:], in0=mex[:, 2:4], in1=var[:], op=ALU.subtract)
            sd = sml.tile([128, 2], F32, name="sd")
            # sd = sqrt(var + EPS): fold the epsilon into the ACT bias
            nc.scalar.activation(sd[:], var[:], AF.Sqrt, bias=eps_sb[:, 0:1], scale=1.0)
            rs = sml.tile([128, 2], F32, name="rs")
            nc.vector.reciprocal(rs[:], sd[:])
            sc = sml.tile([128, 2], F32, name="sc")
            nc.vector.tensor_tensor(out=sc[:], in0=g_sb[:, 2 * bn : 2 * bn + 2], in1=rs[:], op=ALU.mult)
            sh = sml.tile([128, 2], F32, name="sh")
            nc.vector.tensor_tensor(out=sh[:], in0=mean, in1=sc[:], op=ALU.mult)
            nc.vector.tensor_tensor(out=sh[:], in0=be_sb[:, 2 * bn : 2 * bn + 2], in1=sh[:], op=ALU.subtract)
            return sc, sh

        # -------------------------------------------------------------------
        # Residual blocks
        # -------------------------------------------------------------------
        for b in range(NBLOCKS):
            l1, l2 = 2 * b, 2 * b + 1
            # ---- bn1 apply (per chunk so mm1 can start per chunk)
            sc1, sh1 = ag_and_finalize(spack, 2 * b)
            a1 = [chunk_tiles(f"a1_{m}") for m in range(2)]
            spack = sml.tile([128, 4], F32, name="spack", bufs=3)
            sredz = sml.tile([128, 2 * NCH], F32, name="sredz")
            ssredz = sml.tile([128, 2 * NCH], F32, name="ssredz")
            psz = [[None] * NCH for _ in range(2)]
            for ci, (c0, c1) in enumerate(CHUNKS):
                w = c1 - c0
                for m in range(2):
                    nc.scalar.activation(
                        a1[m][ci][:], h[m][ci][:], AF.Relu,
                        bias=sh1[:, m : m + 1], scale=sc1[:, m : m + 1],
                    )
                # ---- mm1 -> z1 in psum; stats of z1 on the fly
                for m in range(2):
                    pz = ps.tile([128, 512], F32, tag="ps", name="pz1")
                    psz[m][ci] = pz
                    for k in range(2):
                        nc.tensor.matmul(
                            pz[:, :w],
                            lhsT=w_sb[l1][k][:, m * 128 : (m + 1) * 128],
                            rhs=a1[k][ci][:],
                            start=(k == 0), stop=(k == 1),
                        )
                    nc.vector.reduce_sum(
                        sredz[:, m * NCH + ci : m * NCH + ci + 1], pz[:, :w],
                        axis=mybir.AxisListType.X,
                    )
                    scr = act.tile([128, 512], F32, name="sqscr5", bufs=3)
                    nc.scalar.activation(
                        scr[:, :w], pz[:, :w], AF.Square,
                        accum_out=ssredz[:, m * NCH + ci : m * NCH + ci + 1],
                    )
            for m in range(2):
                nc.vector.reduce_sum(
                    spack[:, m : m + 1], sredz[:, m * NCH : (m + 1) * NCH],
                    axis=mybir.AxisListType.X,
                )
                nc.vector.reduce_sum(
                    spack[:, 2 + m : 3 + m], ssredz[:, m * NCH : (m + 1) * NCH],
                    axis=mybir.AxisListType.X,
                )
            # ---- bn2 apply (from psum) + mm2 + residual
            sc2, sh2 = ag_and_finalize(spack, 2 * b + 1)
            spack_hb = spack_h  # local S of this block's input h (cols 0:2)
            a2 = [chunk_tiles(f"a2_{m}") for m in range(2)]
            hn = [chunk_tiles(f"h{m}", dtype=F32, bufs=3) for m in range(2)]
            spack = sml.tile([128, 4], F32, name="spack", bufs=3)
            sa2 = sml.tile([128, 2 * NCH], F32, name="sa2")
            ssredh = sml.tile([128, 2 * NCH], F32, name="ssredh")
            for ci, (c0, c1) in enumerate(CHUNKS):
                w = c1 - c0
                for m in range(2):
                    nc.scalar.activation(
                        a2[m][ci][:], psz[m][ci][:, :w], AF.Relu,
                        bias=sh2[:, m : m + 1], scale=sc2[:, m : m + 1],
                        accum_out=sa2[:, m * NCH + ci : m * NCH + ci + 1],
                    )
                for m in range(2):
                    ph = ps.tile([128, 512], F32, tag="ps", name="ph")
                    for k in range(2):
                        nc.tensor.matmul(
                            ph[:, :w],
                            lhsT=w_sb[l2][k][:, m * 128 : (m + 1) * 128],
                            rhs=a2[k][ci][:],
                            start=(k == 0), stop=(k == 1),
                        )
                    # residual add on DVE (h is plain f32, never a PE operand)
                    nc.vector.tensor_tensor(
                        out=hn[m][ci][:], in0=h[m][ci][:], in1=ph[:, :w],
                        op=ALU.add,
                    )
                    scr = act.tile([128, 512], F32, name="sqscr5", bufs=3)
                    if m == 1 and ci < 2:
                        # ACT is the bottleneck here; DVE has slack
                        nc.vector.tensor_tensor(
                            out=scr[:, :w], in0=hn[m][ci][:], in1=hn[m][ci][:],
                            op=ALU.mult,
                        )
                        nc.vector.reduce_sum(
                            ssredh[:, m * NCH + ci : m * NCH + ci + 1],
                            scr[:, :w], axis=mybir.AxisListType.X,
                        )
                    else:
                        nc.scalar.activation(
                            scr[:, :w], hn[m][ci][:], AF.Square,
                            accum_out=ssredh[:, m * NCH + ci : m * NCH + ci + 1],
                        )
            # S(h + z2) = S(h)_local + W2 @ S(a2): the matvec is 4 tiny fp32
            # matmuls on the relu accumulator sums (linearity of row sums)
            sa2k = sml.tile([128, 2], F32, name="sa2k")
            for k in range(2):
                nc.vector.reduce_sum(
                    sa2k[:, k : k + 1], sa2[:, k * NCH : (k + 1) * NCH],
                    axis=mybir.AxisListType.X,
                )
            for m in range(2):
                pv = ps.tile([128, 512], F32, tag="ps", name="pv")
                for k in range(2):
                    nc.tensor.matmul(
                        pv[:, 0:1],
                        lhsT=w_sb[l2][k][:, m * 128 : (m + 1) * 128].bitcast(F32),
                        rhs=sa2k[:, k : k + 1],
                        start=(k == 0), stop=(k == 1),
                    )
                nc.vector.tensor_tensor(
                    out=spack[:, m : m + 1], in0=spack_hb[:, m : m + 1],
                    in1=pv[:, 0:1], op=ALU.add,
                )
                nc.vector.reduce_sum(
                    spack[:, 2 + m : 3 + m], ssredh[:, m * NCH : (m + 1) * NCH],
                    axis=mybir.AxisListType.X,
                )
            h = hn
            spack_h = spack

        # -------------------------------------------------------------------
        # Final BN + output layer + scatter
        # -------------------------------------------------------------------
        scf, shf = ag_and_finalize(spack, 4)
        af = [chunk_tiles(f"af_{m}") for m in range(2)]
        for ci in range(NCH):
            for m in range(2):
                nc.scalar.activation(
                    af[m][ci][:], h[m][ci][:], AF.Relu,
                    bias=shf[:, m : m + 1], scale=scf[:, m : m + 1],
                )

        for t, (t0, t1) in enumerate(PTILES):
            w = t1 - t0
            ci = t0 // 512
            off = t0 - ci * 512
            pp = ps.tile([128, 2 * GG], F32, tag="ps", name="pp")
            for k in range(2):
                nc.tensor.matmul(
                    pp[:w, :],
                    lhsT=af[k][ci][:, off : off + w],
                    rhs=wout_sb[k][:],
                    start=(k == 0), stop=(k == 1),
                )
            pred = act.tile([128, 2 * GG], F32, name="pred", bufs=4)
            nc.vector.tensor_tensor(
                out=pred[:w, :], in0=pp[:w, :], in1=bout_sb[:w, :], op=ALU.add
            )
            nc.gpsimd.indirect_dma_start(
                out=d_out[0:128, :],
                out_offset=bass.IndirectOffsetOnAxis(
                    ap=uidx_sb[0:w, t : t + 1], axis=0
                ),
                in_=pred[0:w, 0:GG],
                in_offset=None,
                bounds_check=OUTROWS - 1,
                oob_is_err=False,
            )
            nc.gpsimd.indirect_dma_start(
                out=d_out[0:128, :],
                out_offset=bass.IndirectOffsetOnAxis(
                    ap=lidx_sb[0:w, t : t + 1], axis=0
                ),
                in_=pred[0:w, GG : 2 * GG],
                in_offset=None,
                bounds_check=OUTROWS - 1,
                oob_is_err=False,
            )

    _pipeline_scatter_waits(nc)
    _split_multi_waits(nc)
    return nc


_NC_CACHE = None


def _get_nc():
    global _NC_CACHE
    if _NC_CACHE is None:
        _NC_CACHE = _build_nc()
    return _NC_CACHE


def kernel(x, w_in, b_in, g1, beta1, w1, bias1, g2, beta2, w2, bias2,
           fg, fbeta, w_out, b_out):
    from concourse.bass_utils import run_bass_kernel_spmd

    x = np.asarray(x, np.float32)
    w_in = np.asarray(w_in, np.float32)
    b_in = np.asarray(b_in, np.float32)
    w1 = np.asarray(w1, np.float32)
    w2 = np.asarray(w2, np.float32)
    w_out = np.asarray(w_out, np.float32)
    b_out = np.asarray(b_out, np.float32)

    # bias1/bias2 are never materialized on device: a per-feature constant
    # shift of the residual stream cancels exactly inside every BatchNorm
    # (with h_true = h_core + c, mean_true = mean_core + c, so
    # bn(h_true) = (h_core - mean_core)*s + beta), and nothing downstream of
    # the final BN sees h directly.

    perm = np.arange(GG).reshape(GRID, GRID).T.reshape(-1)
    w_out_cat = np.concatenate([w_out.T, w_out[perm].T], axis=1)  # [256, 162]
    b_out_cat = np.concatenate([b_out, b_out[perm]])              # [162]
    bout_bc = np.tile(b_out_cat[None, :], (128, 1)).astype(np.float32)

    wtrunk = np.stack([w1[0].T, w2[0].T, w1[1].T, w2[1].T]).astype(np.float32)
    gammas = np.stack([g1[0], g2[0], g1[1], g2[1], fg]).astype(np.float32)
    betas = np.stack([beta1[0], beta2[0], beta1[1], beta2[1], fbeta]).astype(np.float32)

    shared = {
        "winT": np.ascontiguousarray(w_in.T),
        "wtrunk": np.ascontiguousarray(wtrunk),
        "woutT": np.ascontiguousarray(w_out_cat),
        "b_in": np.ascontiguousarray(b_in.reshape(HID, 1)),
        "gammas": gammas,
        "betas": betas,
        "bout_bc": bout_bc,
    }

    in_maps = []
    for c in range(NCORES):
        b, half = c // 2, c % 2
        sl = slice(half * RC, (half + 1) * RC)
        ti, tj = _TI[sl], _TJ[sl]
        rr = np.concatenate([x[b][ti], x[b][tj]], axis=-1)  # [1620, 4]
        uidx = np.full(13 * 128, OUTROWS + 10, np.int32)
        lidx = np.full(13 * 128, OUTROWS + 10, np.int32)
        uidx[:RC] = ti * V + tj
        lidx[:RC] = tj * V + ti
        didx = np.full(128, OUTROWS + 10, np.int32)
        if half == 0:
            dvals = np.arange(0, 24) * (V + 1)
        else:
            dvals = np.arange(24, V) * (V + 1)
        didx[: len(dvals)] = dvals
        in_maps.append({
            "rrT": np.ascontiguousarray(rr.T),
            "uidx": uidx.reshape(13, 128),
            "lidx": lidx.reshape(13, 128),
            "diag_idx": didx.reshape(128, 1),
            **shared,
        })

    global _LAST_IN_MAPS
    _LAST_IN_MAPS = in_maps
    nc = _get_nc()
    res = run_bass_kernel_spmd(nc, in_maps, core_ids=list(range(NCORES)))

    full = np.zeros((BS, V, V, GRID, GRID), np.float32)
    for b in range(BS):
        A = res.results[2 * b]["out"].reshape(V, V, GRID, GRID)
        B = res.results[2 * b + 1]["out"].reshape(V, V, GRID, GRID)
        full[b] = A
        full[b, 24:, 24:] = B[24:, 24:]
        full[b, 23, 57:] = B[23, 57:]
        full[b, 57:, 23] = B[57:, 23]
    return full


# revision 31
# speedup vs baseline: 1.0076x; 1.0076x over previous
"""Trainium2 Bass kernel for nn_Net_58428735095641 (pairwise ResMLP + symmetric
scatter, training-mode BatchNorm with global batch stats).

Sharding: 8 cores = 4 batches x 2 halves of the 3240 pair rows (split at pair
1620). Each core runs the trunk on its 1620 rows in feature-major (transposed)
layout; the 5 BatchNorms need global row statistics, exchanged as raw
(sum, sumsq) via a tiny 8-core AllGather each. The final layer produces both
the 9x9 grid and its transpose pair-major, scattered into a per-core
(81*81, 81) output buffer with indirect DMAs; the host overlays the two
half-buffers per batch with plain slice assignments.
"""
import os
from contextlib import ExitStack

import numpy as np

import concourse.bass as bass
import concourse.mybir as mybir
import concourse.tile as tile
from concourse.vector_clock import ScopedClock

# ---------------------------------------------------------------------------
# Workarounds: the neuronxcc walrus on this path accepts at most ONE sync-wait
# per instruction; split extras onto same-engine nops.
# ---------------------------------------------------------------------------
MAX_WAITS = 1


def _drain_and_barrier(self, tick_clock, wait_clock):
    drain_inst = self.nc.sync.drain()
    wait_clock.add_sem_waits(
        drain_inst.ins, ScopedClock({None: tick_clock.global_clock})
    )
    waits = list(drain_inst.ins.sync_info.on_wait)
    if len(waits) > MAX_WAITS:
        drain_inst.ins.sync_info.on_wait = waits[:MAX_WAITS]
        for i in range(MAX_WAITS, len(waits), MAX_WAITS):
            nop = self.nc.sync.nop(hint="drain_wait_spill", nofuse=True)
            nop.ins.sync_info = mybir.SyncInfo(
                on_wait=waits[i : i + MAX_WAITS], on_update=[]
            )
    self.nc.all_engine_barrier()
    assert self.sems is not None
    popped = self.nc._tile_sem_poison_stack.pop()
    assert popped is self._sem_poison
    self.nc.clear_and_free_semaphores(list(self.sems.allocated().values()))
    self.nc.all_engine_barrier()


tile.TileContext._drain_and_barrier = _drain_and_barrier


def _split_multi_waits(nc):
    for fn in nc.m.functions:
        for bb in fn.blocks:
            insts = list(bb.instructions)
            out = []
            changed = False
            for inst in insts:
                si = inst.sync_info
                if si is not None and si.on_wait and len(si.on_wait) > MAX_WAITS:
                    waits = list(si.on_wait)
                    for j in range(MAX_WAITS, len(waits), MAX_WAITS):
                        out.append(
                            mybir.InstNoOp(
                                name=f"{inst.name}-wsplit{j}",
                                engine=inst.engine,
                                bass_nofuse=True,
                                sync_info=mybir.SyncInfo(
                                    on_wait=waits[j : j + MAX_WAITS], on_update=[]
                                ),
                            )
                        )
                    si.on_wait = waits[:MAX_WAITS]
                    changed = True
                out.append(inst)
            if changed:
                bb.instructions[:] = out


def _pipeline_scatter_waits(nc, depth=7):
    """The 27 indirect scatter DMAs write provably disjoint cells of the output,
    but Tile serializes them (WAW on the output tensor). Replace each op's
    DMASW-chain wait with its sibling's data-ready wait, keeping only a
    ring-capacity pipeline: op n waits for op n-depth's completion so at most
    `depth` ops (depth*128 descriptors <= the 1024-descriptor SWDGE ring) are
    in flight."""
    import copy as _copy

    idmas = []
    for fn in nc.m.functions:
        for bb in fn.blocks:
            for inst in bb.instructions:
                if (
                    type(inst).__name__ == "InstDMACopy"
                    and inst.engine == mybir.EngineType.Pool
                ):
                    idmas.append(inst)
    if not idmas:
        return
    lane_cum = {}
    hist = []
    for inst in idmas:
        upd = inst.sync_info.on_update[0]
        lane_cum[upd.ant_name] = lane_cum.get(upd.ant_name, 0) + upd.update_value
        hist.append((upd.ant_name, upd.id, lane_cum[upd.ant_name]))
    last_data = None
    for n, inst in enumerate(idmas):
        waits = [
            w for w in inst.sync_info.on_wait if not w.ant_name.startswith("DMASW")
        ]
        if waits:
            last_data = waits
        elif last_data is not None:
            waits = [_copy.copy(w) for w in last_data]
        if n >= depth:
            name, sid, val = hist[n - depth]
            w = _copy.copy((last_data or list(inst.sync_info.on_wait))[0])
            w.ant_name = name
            w.id = sid
            w.wait_value = val
            w.wait_mode = "sem-ge-imm"
            waits.append(w)
        inst.sync_info.on_wait = waits


# ---------------------------------------------------------------------------
# Problem constants (hardcoded from the reference)
# ---------------------------------------------------------------------------
GRID = 9
V = 81
NB_FT = 2
HID = 256
NBLOCKS = 2
EPS = 1e-5
BS = 4
NPAIR = 3240          # V*(V-1)/2
RC = 1620             # pair rows per core (half a batch)
NT = BS * NPAIR       # 12960 global rows for BN stats
NCORES = 8
GG = GRID * GRID      # 81
OUTROWS = V * V       # 6561 rows of 81 floats per batch

CHUNKS = [(0, 512), (512, 1024), (1024, 1536), (1536, 1620)]
NCH = len(CHUNKS)
PTILES = [(t * 128, min((t + 1) * 128, RC)) for t in range(13)]

F32 = mybir.dt.float32
F32R = mybir.dt.float32r
I32 = mybir.dt.int32
AF = mybir.ActivationFunctionType
ALU = mybir.AluOpType

_TI, _TJ = np.triu_indices(V, 1)


def _build_nc():
    nc = bass.Bass(num_devices=NCORES, num_swdge_queues=4)

    # fp32r inputs (matmul operands)
    d_rr = nc.dram_tensor("rrT", [4, RC], F32R, kind="ExternalInput")
    d_win = nc.dram_tensor("winT", [4, HID], F32R, kind="ExternalInput")
    d_wtr = nc.dram_tensor("wtrunk", [4, HID, HID], F32R, kind="ExternalInput")
    d_wout = nc.dram_tensor("woutT", [HID, 2 * GG], F32R, kind="ExternalInput")
    # fp32 params
    d_bin = nc.dram_tensor("b_in", [HID, 1], F32, kind="ExternalInput")
    d_g = nc.dram_tensor("gammas", [5, HID], F32, kind="ExternalInput")
    d_b = nc.dram_tensor("betas", [5, HID], F32, kind="ExternalInput")
    d_bout = nc.dram_tensor("bout_bc", [128, 2 * GG], F32, kind="ExternalInput")
    # scatter index tables
    d_uidx = nc.dram_tensor("uidx", [13, 128], I32, kind="ExternalInput")
    d_lidx = nc.dram_tensor("lidx", [13, 128], I32, kind="ExternalInput")
    d_didx = nc.dram_tensor("diag_idx", [128, 1], I32, kind="ExternalInput")
    # output
    d_out = nc.dram_tensor("out", [OUTROWS, GG], F32, kind="ExternalOutput")

    with tile.TileContext(nc) as tc, ExitStack() as ctx:
        cst = ctx.enter_context(tc.tile_pool(name="cst", bufs=1))
        act = ctx.enter_context(tc.tile_pool(name="act", bufs=2))
        sml = ctx.enter_context(tc.tile_pool(name="sml", bufs=2))
        ps = ctx.enter_context(tc.tile_pool(name="ps", bufs=8, space="PSUM"))
        dram = ctx.enter_context(tc.tile_pool(name="dram", bufs=2, space="DRAM"))

        # ---- inputs on the critical path to the first collective come first
        rr_sb = cst.tile([4, RC], F32R)
        nc.sync.dma_start(rr_sb[:], d_rr[:])
        win_sb = cst.tile([4, HID], F32R)
        nc.sync.dma_start(win_sb[:], d_win[:])
        bin_sb = cst.tile([128, 2], F32)
        nc.sync.dma_start(bin_sb[:], d_bin[:].rearrange("(m p) o -> p (m o)", p=128))
        g_sb = cst.tile([128, 10], F32)
        nc.sync.dma_start(g_sb[:], d_g[:].rearrange("n (m p) -> p (n m)", p=128))
        be_sb = cst.tile([128, 10], F32)
        nc.sync.dma_start(be_sb[:], d_b[:].rearrange("n (m p) -> p (n m)", p=128))

        def chunk_tiles(prefix, dtype=F32R, bufs=2):
            return [
                act.tile([128, c1 - c0], dtype, name=f"{prefix}_{ci}", bufs=bufs)
                for ci, (c0, c1) in enumerate(CHUNKS)
            ]

        # -------------------------------------------------------------------
        # Input layer: h0 = relu(W_in @ rrT + b_in); S accumulated by the relu
        # -------------------------------------------------------------------
        h = [chunk_tiles(f"h{m}", dtype=F32, bufs=3) for m in range(2)]
        spack0 = sml.tile([128, 4], F32, name="spack", bufs=3)
        sred = sml.tile([128, 2 * NCH], F32, name="sred")
        ssred = sml.tile([128, 2 * NCH], F32, name="ssred")
        for m in range(2):
            for ci, (c0, c1) in enumerate(CHUNKS):
                w = c1 - c0
                pz = ps.tile([128, 512], F32, tag="ps", name="pz")
                nc.tensor.matmul(
                    pz[:, :w],
                    lhsT=win_sb[:, m * 128 : (m + 1) * 128],
                    rhs=rr_sb[:, c0:c1],
                    start=True, stop=True,
                )
                nc.scalar.activation(
                    h[m][ci][:], pz[:, :w], AF.Relu,
                    bias=bin_sb[:, m : m + 1], scale=1.0,
                    accum_out=sred[:, m * NCH + ci : m * NCH + ci + 1],
                )
                scr = act.tile([128, 512], F32, name="sqscr5", bufs=3)
                if m == 0 and ci >= 2:
                    nc.scalar.activation(
                        scr[:, :w], h[m][ci][:], AF.Square,
                        accum_out=ssred[:, m * NCH + ci : m * NCH + ci + 1],
                    )
                else:
                    # DVE is idle during the input layer: square+reduce there
                    nc.vector.tensor_tensor(
                        out=scr[:, :w], in0=h[m][ci][:], in1=h[m][ci][:],
                        op=ALU.mult,
                    )
                    nc.vector.reduce_sum(
                        ssred[:, m * NCH + ci : m * NCH + ci + 1], scr[:, :w],
                        axis=mybir.AxisListType.X,
                    )
        for m in range(2):
            nc.vector.reduce_sum(
                spack0[:, m : m + 1], sred[:, m * NCH : (m + 1) * NCH],
                axis=mybir.AxisListType.X,
            )
            nc.vector.reduce_sum(
                spack0[:, 2 + m : 3 + m], ssred[:, m * NCH : (m + 1) * NCH],
                axis=mybir.AxisListType.X,
            )
        spack = spack0
        spack_h = spack0

        # ---- remaining constants (needed only after the first collective)
        w_sb = []  # [layer 0..3][kblock 0..1] -> [128, 256]
        for l in range(4):
            row = []
            for k in range(2):
                t = cst.tile([128, HID], F32R, name=f"w{l}{k}")
                nc.sync.dma_start(t[:], d_wtr[l, k * 128 : (k + 1) * 128, :])
                row.append(t)
            w_sb.append(row)
        wout_sb = []
        for k in range(2):
            t = cst.tile([128, 2 * GG], F32R, name=f"wout{k}")
            nc.sync.dma_start(t[:], d_wout[k * 128 : (k + 1) * 128, :])
            wout_sb.append(t)
        bout_sb = cst.tile([128, 2 * GG], F32)
        nc.sync.dma_start(bout_sb[:], d_bout[:])
        uidx_sb = cst.tile([128, 13], I32)
        nc.sync.dma_start(uidx_sb[:], d_uidx[:].rearrange("t p -> p t"))
        lidx_sb = cst.tile([128, 13], I32)
        nc.sync.dma_start(lidx_sb[:], d_lidx[:].rearrange("t p -> p t"))
        didx_sb = cst.tile([128, 1], I32)
        nc.sync.dma_start(didx_sb[:], d_didx[:])
        zer_sb = cst.tile([128, GG], F32)
        nc.vector.memset(zer_sb[:], 0.0)
        eps_sb = cst.tile([128, 1], F32)
        nc.vector.memset(eps_sb[:], EPS)
        # diagonal zero blocks depend only on constants: scatter them now,
        # while the Q7/SWDGE path is otherwise idle
        nc.gpsimd.indirect_dma_start(
            out=d_out[0:128, :],
            out_offset=bass.IndirectOffsetOnAxis(ap=didx_sb[:, 0:1], axis=0),
            in_=zer_sb[:],
            in_offset=None,
            bounds_check=OUTROWS - 1,
            oob_is_err=False,
        )

        # -------------------------------------------------------------------
        # Stats exchange + finalize. spack cols: (S_m0, S_m1, SS_m0, SS_m1).
        # -------------------------------------------------------------------
        def ag_and_finalize(spack, bn):
            """AllGather local (S, SS) and produce per-feature scale/shift."""
            agin = dram.tile([128, 4], F32, name="agin")
            agout = dram.tile([NCORES * 128, 4], F32, name="agout",
                              addr_space="Shared")
            nc.sync.dma_start(agin[:], spack[:])
            nc.gpsimd.collective_compute(
                "AllGather",
                ALU.bypass,
                replica_groups=[list(range(NCORES))],
                ins=[agin[:].opt()],
                outs=[agout[:].opt()],
            )
            gath = sml.tile([128, 32], F32, name="gath")
            nc.sync.dma_start(
                gath[:].rearrange("p (c r) -> p c r", r=8),
                agout[:].rearrange("(r p) c -> p c r", p=128),
            )
            gsum = sml.tile([128, 4], F32, name="gsum")
            nc.vector.reduce_sum(
                gsum[:], gath[:].rearrange("p (c r) -> p c r", r=8),
                axis=mybir.AxisListType.X,
            )
            # mean|ex2 in one op: cols (0:2)=S/NT, (2:4)=SS/NT
            mex = sml.tile([128, 4], F32, name="mex")
            nc.vector.tensor_scalar_mul(mex[:], gsum[:], 1.0 / NT)
            mean = mex[:, 0:2]
            var = sml.tile([128, 2], F32, name="var")
            nc.vector.tensor_tensor(out=var[:], in0=mean, in1=mean, op=ALU.mult)
            # var = ex2 - mean^2 + EPS via tensor_scalar two-op form
            nc.vector.tensor_tensor(out=var[:], in0=mex[:, 2:4], in1=var[:], op=ALU.subtract)
            sd = sml.tile([128, 2], F32, name="sd")
            # sd = sqrt(var + EPS): fold the epsilon into the ACT bias
            nc.scalar.activation(sd[:], var[:], AF.Sqrt, bias=eps_sb[:, 0:1], scale=1.0)
            rs = sml.tile([128, 2], F32, name="rs")
            nc.vector.reciprocal(rs[:], sd[:])
            sc = sml.tile([128, 2], F32, name="sc")
            nc.vector.tensor_tensor(out=sc[:], in0=g_sb[:, 2 * bn : 2 * bn + 2], in1=rs[:], op=ALU.mult)
            sh = sml.tile([128, 2], F32, name="sh")
            nc.vector.tensor_tensor(out=sh[:], in0=mean, in1=sc[:], op=ALU.mult)
            nc.vector.tensor_tensor(out=sh[:], in0=be_sb[:, 2 * bn : 2 * bn + 2], in1=sh[:], op=ALU.subtract)
            return sc, sh

        # -------------------------------------------------------------------
        # Residual blocks
        # -------------------------------------------------------------------
        for b in range(NBLOCKS):
            l1, l2 = 2 * b, 2 * b + 1
            # ---- bn1 apply (per chunk so mm1 can start per chunk)
            sc1, sh1 = ag_and_finalize(spack, 2 * b)
            a1 = [chunk_tiles(f"a1_{m}") for m in range(2)]
            spack = sml.tile([128, 4], F32, name="spack", bufs=3)
            sredz = sml.tile([128, 2 * NCH], F32, name="sredz")
            ssredz = sml.tile([128, 2 * NCH], F32, name="ssredz")
            psz = [[None] * NCH for _ in range(2)]
            for ci, (c0, c1) in enumerate(CHUNKS):
                w = c1 - c0
                for m in range(2):
                    nc.scalar.activation(
                        a1[m][ci][:], h[m][ci][:], AF.Relu,
                        bias=sh1[:, m : m + 1], scale=sc1[:, m : m + 1],
                    )
                # ---- mm1 -> z1 in psum; stats of z1 on the fly
                for m in range(2):
                    pz = ps.tile([128, 512], F32, tag="ps", name="pz1")
                    psz[m][ci] = pz
                    for k in range(2):
                        nc.tensor.matmul(
                            pz[:, :w],
                            lhsT=w_sb[l1][k][:, m * 128 : (m + 1) * 128],
                            rhs=a1[k][ci][:],
                            start=(k == 0), stop=(k == 1),
                        )
                    nc.vector.reduce_sum(
                        sredz[:, m * NCH + ci : m * NCH + ci + 1], pz[:, :w],
                        axis=mybir.AxisListType.X,
                    )
                    scr = act.tile([128, 512], F32, name="sqscr5", bufs=3)
                    nc.scalar.activation(
                        scr[:, :w], pz[:, :w], AF.Square,
                        accum_out=ssredz[:, m * NCH + ci : m * NCH + ci + 1],
                    )
            for m in range(2):
                nc.vector.reduce_sum(
                    spack[:, m : m + 1], sredz[:, m * NCH : (m + 1) * NCH],
                    axis=mybir.AxisListType.X,
                )
                nc.vector.reduce_sum(
                    spack[:, 2 + m : 3 + m], ssredz[:, m * NCH : (m + 1) * NCH],
                    axis=mybir.AxisListType.X,
                )
            # ---- bn2 apply (from psum) + mm2 + residual
            sc2, sh2 = ag_and_finalize(spack, 2 * b + 1)
            spack_hb = spack_h  # local S of this block's input h (cols 0:2)
            a2 = [chunk_tiles(f"a2_{m}") for m in range(2)]
            hn = [chunk_tiles(f"h{m}", dtype=F32, bufs=3) for m in range(2)]
            spack = sml.tile([128, 4], F32, name="spack", bufs=3)
            sa2 = sml.tile([128, 2 * NCH], F32, name="sa2")
            ssredh = sml.tile([128, 2 * NCH], F32, name="ssredh")
            for ci, (c0, c1) in enumerate(CHUNKS):
                w = c1 - c0
                for m in range(2):
                    nc.scalar.activation(
                        a2[m][ci][:], psz[m][ci][:, :w], AF.Relu,
                        bias=sh2[:, m : m + 1], scale=sc2[:, m : m + 1],
                        accum_out=sa2[:, m * NCH + ci : m * NCH + ci + 1],
                    )
                for m in range(2):
                    ph = ps.tile([128, 512], F32, tag="ps", name="ph")
                    for k in range(2):
                        nc.tensor.matmul(
                            ph[:, :w],
                            lhsT=w_sb[l2][k][:, m * 128 : (m + 1) * 128],
                            rhs=a2[k][ci][:],
                            start=(k == 0), stop=(k == 1),
                        )
                    # residual add on DVE (h is plain f32, never a PE operand)
                    nc.vector.tensor_tensor(
                        out=hn[m][ci][:], in0=h[m][ci][:], in1=ph[:, :w],
                        op=ALU.add,
                    )
                    scr = act.tile([128, 512], F32, name="sqscr5", bufs=3)
                    if m == 1 and ci < 2:
                        # ACT is the bottleneck here; DVE has slack
                        nc.vector.tensor_tensor(
                            out=scr[:, :w], in0=hn[m][ci][:], in1=hn[m][ci][:],
                            op=ALU.mult,
                        )
                        nc.vector.reduce_sum(
                            ssredh[:, m * NCH + ci : m * NCH + ci + 1],
                            scr[:, :w], axis=mybir.AxisListType.X,
                        )
                    else:
                        nc.scalar.activation(
                            scr[:, :w], hn[m][ci][:], AF.Square,
                            accum_out=ssredh[:, m * NCH + ci : m * NCH + ci + 1],
                        )
            # S(h + z2) = S(h)_local + W2 @ S(a2): the matvec is 4 tiny fp32
            # matmuls on the relu accumulator sums (linearity of row sums)
            sa2k = sml.tile([128, 2], F32, name="sa2k")
            for k in range(2):
                nc.vector.reduce_sum(
                    sa2k[:, k : k + 1], sa2[:, k * NCH : (k + 1) * NCH],
                    axis=mybir.AxisListType.X,
                )
            for m in range(2):
                pv = ps.tile([128, 512], F32, tag="ps", name="pv")
                for k in range(2):
                    nc.tensor.matmul(
                        pv[:, 0:1],
                        lhsT=w_sb[l2][k][:, m * 128 : (m + 1) * 128].bitcast(F32),
                        rhs=sa2k[:, k : k + 1],
                        start=(k == 0), stop=(k == 1),
                    )
                nc.vector.tensor_tensor(
                    out=spack[:, m : m + 1], in0=spack_hb[:, m : m + 1],
                    in1=pv[:, 0:1], op=ALU.add,
                )
                nc.vector.reduce_sum(
                    spack[:, 2 + m : 3 + m], ssredh[:, m * NCH : (m + 1) * NCH],
                    axis=mybir.AxisListType.X,
                )
            h = hn
            spack_h = spack

        # -------------------------------------------------------------------
        # Final BN + output layer + scatter
        # -------------------------------------------------------------------
        scf, shf = ag_and_finalize(spack, 4)
        af = [chunk_tiles(f"af_{m}") for m in range(2)]
        for ci in range(NCH):
            for m in range(2):
                nc.scalar.activation(
                    af[m][ci][:], h[m][ci][:], AF.Relu,
                    bias=shf[:, m : m + 1], scale=scf[:, m : m + 1],
                )

        for t, (t0, t1) in enumerate(PTILES):
            w = t1 - t0
            ci = t0 // 512
            off = t0 - ci * 512
            pp = ps.tile([128, 2 * GG], F32, tag="ps", name="pp")
            for k in range(2):
                nc.tensor.matmul(
                    pp[:w, :],
                    lhsT=af[k][ci][:, off : off + w],
                    rhs=wout_sb[k][:],
                    start=(k == 0), stop=(k == 1),
                )
            pred = act.tile([128, 2 * GG], F32, name="pred", bufs=4)
            nc.vector.tensor_tensor(
                out=pred[:w, :], in0=pp[:w, :], in1=bout_sb[:w, :], op=ALU.add
            )
            nc.gpsimd.indirect_dma_start(
                out=d_out[0:128, :],
                out_offset=bass.IndirectOffsetOnAxis(
                    ap=uidx_sb[0:w, t : t + 1], axis=0
                ),
                in_=pred[0:w, 0:GG],
                in_offset=None,
                bounds_check=OUTROWS - 1,
                oob_is_err=False,
            )
            nc.gpsimd.indirect_dma_start(
                out=d_out[0:128, :],
                out_offset=bass.IndirectOffsetOnAxis(
                    ap=lidx_sb[0:w, t : t + 1], axis=0
                ),
                in_=pred[0:w, GG : 2 * GG],
                in_offset=None,
                bounds_check=OUTROWS - 1,
                oob_is_err=False,
            )

    _pipeline_scatter_waits(nc)
    _split_multi_waits(nc)
    return nc


_NC_CACHE = None


def _get_nc():
    global _NC_CACHE
    if _NC_CACHE is None:
        _NC_CACHE = _build_nc()
    return _NC_CACHE


def kernel(x, w_in, b_in, g1, beta1, w1, bias1, g2, beta2, w2, bias2,
           fg, fbeta, w_out, b_out):
    from concourse.bass_utils import run_bass_kernel_spmd

    x = np.asarray(x, np.float32)
    w_in = np.asarray(w_in, np.float32)
    b_in = np.asarray(b_in, np.float32)
    w1 = np.asarray(w1, np.float32)
    w2 = np.asarray(w2, np.float32)
    w_out = np.asarray(w_out, np.float32)
    b_out = np.asarray(b_out, np.float32)

    # bias1/bias2 are never materialized on device: a per-feature constant
    # shift of the residual stream cancels exactly inside every BatchNorm
    # (with h_true = h_core + c, mean_true = mean_core + c, so
    # bn(h_true) = (h_core - mean_core)*s + beta), and nothing downstream of
    # the final BN sees h directly.

    perm = np.arange(GG).reshape(GRID, GRID).T.reshape(-1)
    w_out_cat = np.concatenate([w_out.T, w_out[perm].T], axis=1)  # [256, 162]
    b_out_cat = np.concatenate([b_out, b_out[perm]])              # [162]
    bout_bc = np.tile(b_out_cat[None, :], (128, 1)).astype(np.float32)

    wtrunk = np.stack([w1[0].T, w2[0].T, w1[1].T, w2[1].T]).astype(np.float32)
    gammas = np.stack([g1[0], g2[0], g1[1], g2[1], fg]).astype(np.float32)
    betas = np.stack([beta1[0], beta2[0], beta1[1], beta2[1], fbeta]).astype(np.float32)

    shared = {
        "winT": np.ascontiguousarray(w_in.T),
        "wtrunk": np.ascontiguousarray(wtrunk),
        "woutT": np.ascontiguousarray(w_out_cat),
        "b_in": np.ascontiguousarray(b_in.reshape(HID, 1)),
        "gammas": gammas,
        "betas": betas,
        "bout_bc": bout_bc,
    }

    in_maps = []
    for c in range(NCORES):
        b, half = c // 2, c % 2
        sl = slice(half * RC, (half + 1) * RC)
        ti, tj = _TI[sl], _TJ[sl]
        rr = np.concatenate([x[b][ti], x[b][tj]], axis=-1)  # [1620, 4]
        uidx = np.full(13 * 128, OUTROWS + 10, np.int32)
        lidx = np.full(13 * 128, OUTROWS + 10, np.int32)
        uidx[:RC] = ti * V + tj
        lidx[:RC] = tj * V + ti
        didx = np.full(128, OUTROWS + 10, np.int32)
        if half == 0:
            dvals = np.arange(0, 24) * (V + 1)
        else:
            dvals = np.arange(24, V) * (V + 1)
        didx[: len(dvals)] = dvals
        in_maps.append({
            "rrT": np.ascontiguousarray(rr.T),
            "uidx": uidx.reshape(13, 128),
            "lidx": lidx.reshape(13, 128),
            "diag_idx": didx.reshape(128, 1),
            **shared,
        })

    global _LAST_IN_MAPS
    _LAST_IN_MAPS = in_maps
    nc = _get_nc()
    res = run_bass_kernel_spmd(nc, in_maps, core_ids=list(range(NCORES)))

    full = np.zeros((BS, V, V, GRID, GRID), np.float32)
    for b in range(BS):
        A = res.results[2 * b]["out"].reshape(V, V, GRID, GRID)
        B = res.results[2 * b + 1]["out"].reshape(V, V, GRID, GRID)
        full[b] = A
        full[b, 24:, 24:] = B[24:, 24:]
        full[b, 23, 57:] = B[23, 57:]
        full[b, 57:, 23] = B[57:, 23]
    return full
